# revision 11
# baseline (speedup 1.0000x reference)
"""CapsLayer2D Trainium2 kernel (8-core SPMD, data-parallel over batch).

Math: per position p (of B*R*C) and capsule n:
  U[n,i,o] = sum_e x[p,i,e] * W[n,i,e,o]          (u_hat)
  b0 = 1/64; 2x { v = squash(sum_i b*U); b += sum_o U*v }; out = squash(sum_i b*U)

Mapping:
  - 8 cores, 2 batches each -> 392 positions/core, 4 pos-blocks of 98.
  - Phase 1: S[p,n,o] = sum_{i,e} x*W as dense K=1024 accumulating matmuls
    (v0 = squash(S/64) since b0 is uniform).
  - Phase 2: per (block, n-pair) unit, u_hat materialized into PSUM via
    block-diagonal-W matmuls (stationary = xT chunk, moving = BD(W), N=256),
    then routing iterations as DVE mul + segmented-reduce ops reading PSUM.
  - Host pre-builds xT (transposed inputs), BD(W), dense W.
"""
import numpy as np

import concourse.bacc as bacc
import concourse.bass as bass
import concourse.mybir as mybir
import concourse.tile as tile
from concourse.bass_utils import run_bass_kernel_spmd

N_CORES = 8
B, R, C = 16, 14, 14
N_IN, D_IN = 64, 16          # i, e
N_CAPS, CAPS_DIM = 10, 16    # n, o
IE = N_IN * D_IN             # 1024
POS = (B // N_CORES) * R * C # 392 positions per core
BLK = 98                     # pos-block size
NBLK = POS // BLK            # 4
NF = N_CAPS // 2             # 5 units of 2 capsules
NCH = IE // 128              # 8 contraction chunks
F32 = mybir.dt.float32

# u_hat matmuls run in bf16 (1 col/cycle at any N; fp32 is 4x slower,
# fp32r needs producer-side rounding the DMA can't provide).
BF16 = mybir.dt.bfloat16


def _squash(nc, pool, s_ap, v_ap, n):
    """v = squash(s): s_ap/v_ap are [98, n, 16] APs; n capsules."""
    P = s_ap.shape[0]
    sq = pool.tile([P, n * 16], F32, tag="sq")
    nc.scalar.activation(sq[:].rearrange("p (n o) -> p n o", o=16), s_ap,
                         mybir.ActivationFunctionType.Square)
    q = pool.tile([P, n], F32, tag="q")
    nc.vector.tensor_reduce(q[:], sq[:].rearrange("p (n o) -> p n o", o=16),
                            axis=mybir.AxisListType.X, op=mybir.AluOpType.add)
    rt = pool.tile([P, n], F32, tag="rt")
    nc.scalar.activation(rt[:], q[:], mybir.ActivationFunctionType.Sqrt)
    qp = pool.tile([P, n], F32, tag="qp")
    nc.vector.tensor_scalar_add(qp[:], q[:], 1.0)
    rc = pool.tile([P, n], F32, tag="rc")
    nc.vector.reciprocal(rc[:], qp[:])
    al = pool.tile([P, n], F32, tag="al")
    nc.vector.tensor_mul(al[:], rt[:], rc[:])
    alb = al[:].unsqueeze(2).broadcast_to([P, n, 16])
    nc.vector.tensor_mul(v_ap, s_ap, alb)


def build_kernel(dbg=False):
    nc = bacc.Bacc("TRN2", target_bir_lowering=False, debug=False,
                   num_devices=N_CORES)
    xT = nc.dram_tensor("xT", [IE, POS], F32, kind="ExternalInput").ap()
    bdw = nc.dram_tensor("bdw", [128, NCH * N_CAPS * 128], BF16,
                         kind="ExternalInput").ap()
    wd = nc.dram_tensor("wd", [IE, N_CAPS * 16], F32, kind="ExternalInput").ap()
    out = nc.dram_tensor("out", [POS, N_CAPS * 16], F32,
                         kind="ExternalOutput").ap()
    if dbg:
        dbg_s0 = nc.dram_tensor("dbg_s0", [BLK, NBLK * 160], F32,
                                kind="ExternalOutput").ap()
        dbg_v0 = nc.dram_tensor("dbg_v0", [BLK, NBLK * 160], F32,
                                kind="ExternalOutput").ap()
        dbg_u = nc.dram_tensor("dbg_u", [BLK, 2048], F32,
                               kind="ExternalOutput").ap()
        dbg_b1 = nc.dram_tensor("dbg_b1", [BLK, 128], F32,
                                kind="ExternalOutput").ap()

    with tile.TileContext(nc) as tc:
        with tc.tile_pool(name="const", bufs=1) as const, \
             tc.tile_pool(name="work", bufs=3) as work:
            # ---- load inputs ----
            xt_t = const.tile([128, NCH * POS], F32)     # chunk g at cols g*POS
            for g in range(NCH):
                nc.sync.dma_start(xt_t[:, g * POS:(g + 1) * POS],
                                  xT[g * 128:(g + 1) * 128, :])
            bdw_t = const.tile([128, NCH * N_CAPS * 128], BF16)
            nc.sync.dma_start(bdw_t[:], bdw[:])
            wd_t = const.tile([128, NCH * N_CAPS * 16], F32)
            for g in range(NCH):
                nc.sync.dma_start(wd_t[:, g * 160:(g + 1) * 160],
                                  wd[g * 128:(g + 1) * 128, :])

            xtb_t = const.tile([128, NCH * POS], BF16)   # bf16 copy for u_hat
            nc.vector.tensor_copy(xtb_t[:], xt_t[:])

            s0_t = const.tile([BLK, NBLK * 160], F32)    # S/64 per block
            v0_t = const.tile([BLK, NBLK * 160], F32)
            out_t = const.tile([BLK, NBLK * 160], F32)

            # ---- phase 1: S = sum_ie x*W ; v0 = squash(S/64) ----
            with tc.tile_pool(name="psum_s", bufs=4, space="PSUM") as psum_s:
                for b in range(NBLK):
                    for f in range(NF):
                        ps = psum_s.tile([BLK, 32], F32, tag="ps")
                        for g in range(NCH):
                            nc.tensor.matmul(
                                ps[:],
                                xt_t[:, g * POS + b * BLK: g * POS + (b + 1) * BLK],
                                wd_t[:, g * 160 + f * 32: g * 160 + (f + 1) * 32],
                                start=(g == 0), stop=(g == NCH - 1))
                        nc.scalar.activation(
                            s0_t[:, b * 160 + f * 32: b * 160 + (f + 1) * 32],
                            ps[:], mybir.ActivationFunctionType.Copy,
                            scale=1.0 / N_IN)
                for b in range(NBLK):
                    sb = s0_t[:, b * 160:(b + 1) * 160].rearrange(
                        "p (n o) -> p n o", o=16)
                    vb = v0_t[:, b * 160:(b + 1) * 160].rearrange(
                        "p (n o) -> p n o", o=16)
                    _squash(nc, work, sb, vb, N_CAPS)

            # ---- phase 2: u_hat units + routing ----
            with tc.tile_pool(name="psum_u", bufs=2, space="PSUM") as psum_u:
                for b in range(NBLK):
                    for f in range(NF):
                        up = psum_u.tile([BLK, 2048], F32, tag="up")
                        for g in range(NCH):
                            lhs = xtb_t[:, g * POS + b * BLK: g * POS + (b + 1) * BLK]
                            rhs = bdw_t[:, g * 1280 + f * 256: g * 1280 + (f + 1) * 256]
                            nc.tensor.matmul(
                                up[:, g * 256:(g + 1) * 256], lhs, rhs,
                                start=True, stop=True)
                        # U view [p, g, n2, i8, o]
                        U = up[:].rearrange("p (g n i o) -> p g n i o",
                                            g=8, n=2, i=8, o=16)
                        if dbg and b == 0 and f == 0:
                            ucp = work.tile([BLK, 2048], F32, tag="ucp")
                            nc.vector.tensor_copy(ucp[:], up[:])
                            nc.sync.dma_start(dbg_u[:], ucp[:])
                        bco = work.tile([BLK, 128], F32, tag="bco")  # (n2,g,i8)
                        nc.vector.memset(bco[:], 1.0 / N_IN)
                        v_ap = v0_t[:, b * 160 + f * 32: b * 160 + (f + 1) * 32] \
                            .rearrange("p (n o) -> p n o", o=16)
                        s_t = work.tile([BLK, 32], F32, tag="s_t")
                        v_t = work.tile([BLK, 32], F32, tag="v_t")
                        # Per-capsule views (DVE ISA: max 3 free dims per AP).
                        # U_n(n2): [p, g, i, o] slice of the unit's u_hat.
                        def U_n(n2):
                            return U[:, :, n2, :, :]

                        for it in range(3):
                            # agreement: b += sum_o U * v   (skip on last pass)
                            if it > 0:
                                v_ap = v_t[:].rearrange("p (n o) -> p n o", o=16)
                            if it < 2:
                                agr = work.tile([BLK, 128], F32, tag="agr")
                                for n2 in range(2):
                                    P = work.tile([BLK, 1024], F32, tag="P")
                                    Pv = P[:].rearrange(
                                        "p (g i o) -> p g i o", g=8, i=8, o=16)
                                    vb = v_ap[:, n2:n2 + 1, :].unsqueeze(2) \
                                        .broadcast_to([BLK, 8, 8, 16])
                                    nc.vector.tensor_mul(Pv, U_n(n2), vb)
                                    nc.vector.tensor_reduce(
                                        agr[:, n2 * 64:(n2 + 1) * 64].rearrange(
                                            "p (g i) -> p g i", i=8),
                                        Pv, axis=mybir.AxisListType.X,
                                        op=mybir.AluOpType.add)
                                nc.vector.tensor_add(bco[:], bco[:], agr[:])
                                if dbg and b == 0 and f == 0 and it == 0:
                                    nc.sync.dma_start(dbg_b1[:], bco[:])
                            # v-sum: s = sum_{g,i8} U * b
                            for n2 in range(2):
                                Q = work.tile([BLK, 1024], F32, tag="Q")
                                Qv = Q[:].rearrange(
                                    "p (g i o) -> p g i o", g=8, i=8, o=16)
                                bb = bco[:, n2 * 64:(n2 + 1) * 64] \
                                    .rearrange("p (g i) -> p g i", i=8) \
                                    .unsqueeze(3).broadcast_to([BLK, 8, 8, 16])
                                nc.vector.tensor_mul(Qv, U_n(n2), bb)
                                nc.vector.tensor_reduce(
                                    s_t[:, n2 * 16:(n2 + 1) * 16],
                                    Qv.rearrange("p g i o -> p o g i"),
                                    axis=mybir.AxisListType.XY,
                                    op=mybir.AluOpType.add)
                            dst = v_t[:] if it < 2 else \
                                out_t[:, b * 160 + f * 32: b * 160 + (f + 1) * 32]
                            _squash(nc, work,
                                    s_t[:].rearrange("p (n o) -> p n o", o=16),
                                    dst.rearrange("p (n o) -> p n o", o=16), 2)

            for b in range(NBLK):
                nc.sync.dma_start(out[b * BLK:(b + 1) * BLK, :],
                                  out_t[:, b * 160:(b + 1) * 160])
            if dbg:
                nc.sync.dma_start(dbg_s0[:], s0_t[:])
                nc.sync.dma_start(dbg_v0[:], v0_t[:])
    nc.compile()
    return nc


def _host_prep(inputs, W):
    """Build per-core input maps from full inputs."""
    x = np.ascontiguousarray(inputs, dtype=np.float32).reshape(B, R * C, IE)
    Wf = np.ascontiguousarray(W, dtype=np.float32)  # [n, i, e, o]
    # bdw[(i8,e), (g,n,i8,o)]
    Wg = Wf.reshape(N_CAPS, 8, 8, D_IN, CAPS_DIM)   # [n, g, i8, e, o]
    bdw6 = np.zeros((8, D_IN, 8, N_CAPS, 8, CAPS_DIM), dtype=np.float32)
    for i8 in range(8):
        # [n, g, e, o] -> [e, g, n, o]
        bdw6[i8, :, :, :, i8, :] = Wg[:, :, i8, :, :].transpose(2, 1, 0, 3)
    import ml_dtypes
    bdw = bdw6.reshape(128, NCH * N_CAPS * 128).astype(ml_dtypes.bfloat16)
    wd = Wf.transpose(1, 2, 0, 3).reshape(IE, N_CAPS * CAPS_DIM)
    bpc = B // N_CORES
    in_maps = []
    for c in range(N_CORES):
        xc = x[c * bpc:(c + 1) * bpc].reshape(POS, IE)
        in_maps.append({
            "xT": np.ascontiguousarray(xc.T),
            "bdw": bdw,
            "wd": wd,
        })
    return in_maps


_NC_CACHE = []


def kernel(inputs: np.ndarray, W: np.ndarray) -> np.ndarray:
    in_maps = _host_prep(inputs, W)
    if not _NC_CACHE:
        _NC_CACHE.append(build_kernel())
    nc = _NC_CACHE[0]
    res = run_bass_kernel_spmd(nc, in_maps, list(range(N_CORES)))
    outs = [res.results[c]["out"] for c in range(N_CORES)]
    full = np.concatenate(outs, axis=0)  # [3136, 160]
    return full.reshape(B, R, C, N_CAPS, CAPS_DIM)
